# revision 36
# baseline (speedup 1.0000x reference)
"""Trainium2 Bass kernel for a K=1 neighborhood-attention block.

Reference computation (per batch b, N=2048 positions, C=512 channels):
    Q  = x @ Wq^T + bq ;  K = x @ Wk^T + bk ;  V = x @ Wv^T + bv
    s[n]   = Q[n] . K[nbr[n]] + rel_bias[0,0]
    scores = one-hot-sparse [N, N]: row n has s[n] at column nbr[n], zeros else
    probs  = softmax(scores / sqrt(C))
    out    = probs @ V[nbr] ;  y = out @ Wo^T + bo

Because each score row is all-zeros except one entry, softmax collapses
to per-row scalars w0 = 1/(e^t + N-1), w1 = 1 - N*w0 applied to two
dense GEMMs (weight folding A = Wq^T Wk, B = Wv^T Wo^T):
    s[n] = x[n] A xg[n]^T + (bias terms);  XB = x @ B
    y[n] = w1[n] * (XB[nbr2[n]] - mean(xg) @ B + beta...) + S''/N

The O(N*C^2) work — the two GEMMs XA = x @ A and XB = x @ B — runs on
the NeuronCores in fp8 DoubleRow. Everything O(N*C) (gathers, rowdot
score, the collapsed softmax, bias terms, recombination) runs on the
host in f32, which is exact and keeps device traffic at 1.5 MB in /
2 MB out per core — the end-to-end time is chip-HBM-bound with all 8
cores streaming. Device outputs XA and XB in fp8 (overall 2.6e-3).

Per-core program: 64 DoubleRow matmuls + PSUM evacuations batched two
tiles per op (XA on DVE, XB on ACT) + per-2-tile output DMAs spread
over the gpsimd/scalar rings (per-tile drains on sync at the tail).
Head transfers are split ~128KB across all three rings and the head
XA matmuls are phased by contraction half so the PE starts on the
first weight half. All DRAM I/O is pre-tiled host-side (partition dim
first, contiguous per partition) so every DMA is 128 fat descriptors.
Data-parallel over batch: 8 batches on 8 cores, weights replicated.
"""

import os

import numpy as np

# Recover wedged NeuronCores from a previous crashed run at NRT init.
os.environ.setdefault("NEURON_RT_RESET_CORES", "1")

B, N, C = 8, 2048, 512
P = 128
NT = N // P          # 16 n-tiles
KC = C // P          # 4 contraction chunks
FD = 512             # matmul moving free dim / psum bank

# main-matmul dtype: float8e4 (DoubleRow, fastest), bfloat16, float32r, float32
MM_DT = os.environ.get("NAB_MM_DT", "float8e4")

_TRACE = {"enabled": False, "trace_cores": None, "last": None}
_CACHE = {}


def _np_dt(name):
    import ml_dtypes

    return {
        "bfloat16": ml_dtypes.bfloat16,
        "float8e4": ml_dtypes.float8_e4m3,
    }.get(name, np.float32)


def _build_program(mm_dt_str):
    import concourse.tile as tile
    from concourse import bacc, mybir

    mm_dt = getattr(mybir.dt, mm_dt_str)
    f32 = mybir.dt.float32
    dr = mm_dt_str == "float8e4" and os.environ.get("NAB_DR", "1") == "1"
    kstep = 2 if dr else 1
    pmode = mybir.MatmulPerfMode.DoubleRow if dr else None

    nc = bacc.Bacc("TRN2", target_bir_lowering=False, debug=False)

    # DRAM I/O, pre-tiled host-side: partition dim first, per-partition
    # data contiguous.
    xt_d = nc.dram_tensor("xt", [P, NT, KC, P], mm_dt, kind="ExternalInput")
    a_d = nc.dram_tensor("a", [P, KC, C], mm_dt, kind="ExternalInput")
    bm_d = nc.dram_tensor("bm", [P, KC, C], mm_dt, kind="ExternalInput")
    z_d = nc.dram_tensor("z", [P, NT, 2, C], mm_dt, kind="ExternalOutput")

    with tile.TileContext(nc) as tc:
        with (
            tc.tile_pool(name="singles", bufs=1) as singles,
            tc.tile_pool(name="zpair", bufs=6) as zpair_pool,
            tc.tile_pool(name="xa_psum", bufs=2, space="PSUM") as xa_pool,
            tc.tile_pool(name="xb_psum", bufs=2, space="PSUM") as xb_pool,
        ):
            xt_sb = singles.tile([P, NT, KC, P], mm_dt)
            a_sb = singles.tile([P, KC, C], mm_dt)
            bm_sb = singles.tile([P, KC, C], mm_dt)

            xt_ap = xt_d.ap()
            z_ap = z_d.ap()

            # Head transfers split small across three rings so the first
            # matmuls gate on ~128KB each; bulk xt arrives in per-2-tile
            # pieces so the PE rides the stream. Outputs ride the gpsimd
            # ring except the final drain (sync).
            a_ap = a_d.ap()
            bm_ap = bm_d.ap()
            nc.sync.dma_start(xt_sb[:, 0:2], xt_ap[:, 0:2])
            nc.scalar.dma_start(a_sb[:, 0:2], a_ap[:, 0:2])
            nc.gpsimd.dma_start(xt_sb[:, 2:4], xt_ap[:, 2:4])
            nc.scalar.dma_start(a_sb[:, 2:4], a_ap[:, 2:4])
            nc.sync.dma_start(xt_sb[:, 4:6], xt_ap[:, 4:6])
            nc.scalar.dma_start(bm_sb[:, 0:2], bm_ap[:, 0:2])
            nc.gpsimd.dma_start(bm_sb[:, 2:4], bm_ap[:, 2:4])
            nc.scalar.dma_start(xt_sb[:, 6:8], xt_ap[:, 6:8])
            nc.sync.dma_start(xt_sb[:, 8:10], xt_ap[:, 8:10])
            nc.scalar.dma_start(xt_sb[:, 10:12], xt_ap[:, 10:12])
            nc.sync.dma_start(xt_sb[:, 12:14], xt_ap[:, 12:14])
            nc.gpsimd.dma_start(xt_sb[:, 14:16], xt_ap[:, 14:16])

            def mm(psum, pslot, ti, kc, w_sb):
                nc.tensor.matmul(
                    psum[:, pslot, :],
                    xt_sb[:, ti, kc : kc + kstep, :],
                    w_sb[:, kc : kc + kstep, :],
                    start=(kc == 0),
                    stop=(kc + kstep == KC),
                    perf_mode=pmode,
                )

            # pairs of tiles share a 2-bank PSUM tile and one evac op each;
            # head (pairs 0-1): XA phased by contraction pair so tile 0
            # gates on only the first half of a, XB deferred until bm lands
            head = {}
            for pi in range(2):
                z_new = zpair_pool.tile([P, 2, 2, C], mm_dt, tag="zpair")
                xa_new = xa_pool.tile([P, 2, FD], f32, tag="xa")
                xb_new = xb_pool.tile([P, 2, FD], f32, tag="xb")
                head[pi] = (z_new, xa_new, xb_new)
            for kc in range(0, KC, kstep):
                for tj in range(4):
                    mm(head[tj // 2][1], tj % 2, tj, kc, a_sb)
            for pi in range(2):
                z_pair, xa_ps, xb_ps = head[pi]
                nc.vector.tensor_copy(z_pair[:, :, 0, :], xa_ps[:])
            for pi in range(2):
                z_pair, xa_ps, xb_ps = head[pi]
                for j in range(2):
                    for kc in range(0, KC, kstep):
                        mm(xb_ps, j, 2 * pi + j, kc, bm_sb)
                nc.scalar.activation(
                    out=z_pair[:, :, 1, :],
                    in_=xb_ps[:],
                    func=mybir.ActivationFunctionType.Copy,
                )
                [nc.gpsimd, nc.scalar][pi % 2].dma_start(
                    z_ap[:, 2 * pi : 2 * pi + 2], z_pair[:]
                )

            for pi in range(2, NT // 2):
                t0 = 2 * pi
                z_pair = zpair_pool.tile([P, 2, 2, C], mm_dt, tag="zpair")
                xa_ps = xa_pool.tile([P, 2, FD], f32, tag="xa")
                xb_ps = xb_pool.tile([P, 2, FD], f32, tag="xb")
                for j in range(2):
                    for kc in range(0, KC, kstep):
                        mm(xa_ps, j, t0 + j, kc, a_sb)
                nc.vector.tensor_copy(z_pair[:, :, 0, :], xa_ps[:])
                if pi == NT // 2 - 1:
                    # tail: ship the XA halves as soon as the pair CAST is
                    # done, then per-tile XB evacs (ACT/DVE) with 64KB
                    # half-drains so the final transfer is minimal
                    nc.sync.dma_start(z_ap[:, t0, 0:1], z_pair[:, 0, 0:1])
                    nc.sync.dma_start(z_ap[:, t0 + 1, 0:1], z_pair[:, 1, 0:1])
                    for j in range(2):
                        for kc in range(0, KC, kstep):
                            mm(xb_ps, j, t0 + j, kc, bm_sb)
                    nc.scalar.activation(
                        out=z_pair[:, 0, 1, :],
                        in_=xb_ps[:, 0, :],
                        func=mybir.ActivationFunctionType.Copy,
                    )
                    nc.sync.dma_start(z_ap[:, t0, 1:2], z_pair[:, 0, 1:2])
                    nc.vector.tensor_copy(z_pair[:, 1, 1, :], xb_ps[:, 1, :])
                    nc.sync.dma_start(z_ap[:, t0 + 1, 1:2], z_pair[:, 1, 1:2])
                else:
                    for j in range(2):
                        for kc in range(0, KC, kstep):
                            mm(xb_ps, j, t0 + j, kc, bm_sb)
                    nc.scalar.activation(
                        out=z_pair[:, :, 1, :],
                        in_=xb_ps[:],
                        func=mybir.ActivationFunctionType.Copy,
                    )
                    [nc.gpsimd, nc.scalar][pi % 2].dma_start(
                        z_ap[:, t0 : t0 + 2], z_pair[:]
                    )

    nc.compile()
    return nc


def kernel(x, neighbors, Wq, bq, Wk, bk, Wv, bv, rel_bias, Wo, bo):
    from concourse.bass_utils import run_bass_kernel_spmd

    x = np.asarray(x, dtype=np.float32)
    Wq = np.asarray(Wq, dtype=np.float32)
    Wk = np.asarray(Wk, dtype=np.float32)
    Wv = np.asarray(Wv, dtype=np.float32)
    Wo = np.asarray(Wo, dtype=np.float32)
    bq = np.asarray(bq, dtype=np.float32)
    bk = np.asarray(bk, dtype=np.float32)
    bv = np.asarray(bv, dtype=np.float32)
    bo = np.asarray(bo, dtype=np.float32)
    rel_bias = np.asarray(rel_bias, dtype=np.float32)
    nbr = np.asarray(neighbors).reshape(N, -1)[:, 0].astype(np.int64)
    nbr2 = nbr[nbr]

    mm_np = _np_dt(MM_DT)

    # host-side weight folding (tiny)
    A = (Wq.T @ Wk).astype(np.float32)            # [C, C]
    Bm = (Wv.T @ Wo.T).astype(np.float32)         # [C, C]
    beta = (Wo @ bv + bo).astype(np.float32)      # [C]
    u = (Wq.T @ bk).astype(np.float32)
    v = (Wk.T @ bq).astype(np.float32)
    const = float(bq @ bk) + float(rel_bias[0, 0])

    key = MM_DT
    if key not in _CACHE:
        _CACHE[key] = _build_program(key)
    nc = _CACHE[key]

    def tile_T(t):  # [N, C] -> [P, NT, KC, P] (x^T pre-tiled per partition)
        return np.ascontiguousarray(
            t.reshape(NT, P, KC, P).transpose(3, 0, 2, 1)
        )

    A_t = np.ascontiguousarray(A.reshape(KC, P, C).transpose(1, 0, 2)).astype(mm_np)
    Bm_t = np.ascontiguousarray(Bm.reshape(KC, P, C).transpose(1, 0, 2)).astype(mm_np)

    in_maps = [
        {"xt": tile_T(x[b]).astype(mm_np), "a": A_t, "bm": Bm_t} for b in range(B)
    ]

    res = run_bass_kernel_spmd(
        nc,
        in_maps,
        core_ids=list(range(B)),
        trace=_TRACE["enabled"],
        trace_cores=_TRACE["trace_cores"],
    )
    _TRACE["last"] = res

    # host-side O(N*C) epilogue, exact in f32:
    # y[n] = w1[n]*(XB[nbr2[n]] + beta) + w0[n]*S'',  S'' = sxg@B + N*beta
    xg = x[:, nbr, :]                             # [B, N, C]
    sbias = x @ u + xg @ v + const                # [B, N]
    S2pp = xg.sum(axis=1) @ Bm + float(N) * beta  # [B, C] = S''

    y = np.empty((B, N, C), dtype=np.float32)
    for b in range(B):
        z = res.results[b]["z"].astype(np.float32)  # [P, NT, 2, C]
        z = z.transpose(1, 0, 2, 3)                 # [NT, P, 2, C]
        XA = np.ascontiguousarray(z[:, :, 0, :]).reshape(N, C)
        XB = np.ascontiguousarray(z[:, :, 1, :]).reshape(N, C)
        s = np.einsum("nc,nc->n", XA, xg[b]) + sbias[b]
        t = s / np.sqrt(C, dtype=np.float32)
        e = np.exp(t)
        w0 = 1.0 / (e + (N - 1))
        w1 = 1.0 - N * w0
        y[b] = w1[:, None] * (XB[nbr2] + beta[None, :]) + w0[:, None] * S2pp[b][None, :]
    return y


# revision 37
# speedup vs baseline: 1.0522x; 1.0522x over previous
"""Trainium2 Bass kernel for a K=1 neighborhood-attention block.

Reference computation (per batch b, N=2048 positions, C=512 channels):
    Q  = x @ Wq^T + bq ;  K = x @ Wk^T + bk ;  V = x @ Wv^T + bv
    s[n]   = Q[n] . K[nbr[n]] + rel_bias[0,0]
    scores = one-hot-sparse [N, N]: row n has s[n] at column nbr[n], zeros else
    probs  = softmax(scores / sqrt(C))
    out    = probs @ V[nbr] ;  y = out @ Wo^T + bo

Because each score row is all-zeros except one entry, softmax collapses
to per-row scalars w0 = 1/(e^t + N-1), w1 = 1 - N*w0 applied to two
dense GEMMs (weight folding A = Wq^T Wk, B = Wv^T Wo^T):
    s[n] = x[n] A xg[n]^T + (bias terms);  XB = x @ B
    y[n] = w1[n] * (XB[nbr2[n]] - mean(xg) @ B + beta...) + S''/N

The O(N*C^2) work — the two GEMMs XA = x @ A and XB = x @ B — runs on
the NeuronCores in fp8 DoubleRow. Everything O(N*C) (gathers, rowdot
score, the collapsed softmax, bias terms, recombination) runs on the
host in f32, which is exact and keeps device traffic at 1.5 MB in /
2 MB out per core — the end-to-end time is chip-HBM-bound with all 8
cores streaming. Device outputs XA and XB in fp8 (overall 2.6e-3).

Per-core program: 64 DoubleRow matmuls + PSUM evacuations batched two
tiles per op (XA on DVE, XB on ACT) + per-2-tile output DMAs spread
over the gpsimd/scalar rings (per-tile drains on sync at the tail).
Head transfers are split ~128KB across all three rings and the head
XA matmuls are phased by contraction half so the PE starts on the
first weight half. All DRAM I/O is pre-tiled host-side (partition dim
first, contiguous per partition) so every DMA is 128 fat descriptors.
Data-parallel over batch: 8 batches on 8 cores, weights replicated.
"""

import os

import numpy as np

# Recover wedged NeuronCores from a previous crashed run at NRT init.
os.environ.setdefault("NEURON_RT_RESET_CORES", "1")

B, N, C = 8, 2048, 512
P = 128
NT = N // P          # 16 n-tiles
KC = C // P          # 4 contraction chunks
FD = 512             # matmul moving free dim / psum bank

# main-matmul dtype: float8e4 (DoubleRow, fastest), bfloat16, float32r, float32
MM_DT = os.environ.get("NAB_MM_DT", "float8e4")

_TRACE = {"enabled": False, "trace_cores": None, "last": None}
_CACHE = {}


def _np_dt(name):
    import ml_dtypes

    return {
        "bfloat16": ml_dtypes.bfloat16,
        "float8e4": ml_dtypes.float8_e4m3,
    }.get(name, np.float32)


def _build_program(mm_dt_str):
    import concourse.tile as tile
    from concourse import bacc, mybir

    mm_dt = getattr(mybir.dt, mm_dt_str)
    f32 = mybir.dt.float32
    dr = mm_dt_str == "float8e4" and os.environ.get("NAB_DR", "1") == "1"
    kstep = 2 if dr else 1
    pmode = mybir.MatmulPerfMode.DoubleRow if dr else None

    nc = bacc.Bacc("TRN2", target_bir_lowering=False, debug=False)

    # DRAM I/O, pre-tiled host-side: partition dim first, per-partition
    # data contiguous.
    xt_d = nc.dram_tensor("xt", [P, NT, KC, P], mm_dt, kind="ExternalInput")
    a_d = nc.dram_tensor("a", [P, KC, C], mm_dt, kind="ExternalInput")
    bm_d = nc.dram_tensor("bm", [P, KC, C], mm_dt, kind="ExternalInput")
    z_d = nc.dram_tensor("z", [P, NT, 2, C], mm_dt, kind="ExternalOutput")

    with tile.TileContext(nc) as tc:
        with (
            tc.tile_pool(name="singles", bufs=1) as singles,
            tc.tile_pool(name="zpair", bufs=6) as zpair_pool,
            tc.tile_pool(name="xa_psum", bufs=2, space="PSUM") as xa_pool,
            tc.tile_pool(name="xb_psum", bufs=2, space="PSUM") as xb_pool,
        ):
            xt_sb = singles.tile([P, NT, KC, P], mm_dt)
            a_sb = singles.tile([P, KC, C], mm_dt)
            bm_sb = singles.tile([P, KC, C], mm_dt)

            xt_ap = xt_d.ap()
            z_ap = z_d.ap()

            # Head transfers split small across three rings so the first
            # matmuls gate on ~128KB each; bulk xt arrives in per-2-tile
            # pieces so the PE rides the stream. Outputs ride the gpsimd
            # ring except the final drain (sync).
            a_ap = a_d.ap()
            bm_ap = bm_d.ap()
            nc.sync.dma_start(xt_sb[:, 0:2], xt_ap[:, 0:2])
            nc.scalar.dma_start(a_sb[:, 0:2], a_ap[:, 0:2])
            nc.gpsimd.dma_start(xt_sb[:, 2:4], xt_ap[:, 2:4])
            nc.scalar.dma_start(a_sb[:, 2:4], a_ap[:, 2:4])
            nc.sync.dma_start(xt_sb[:, 4:7], xt_ap[:, 4:7])
            nc.scalar.dma_start(bm_sb[:, 0:2], bm_ap[:, 0:2])
            nc.gpsimd.dma_start(bm_sb[:, 2:4], bm_ap[:, 2:4])
            nc.scalar.dma_start(xt_sb[:, 7:10], xt_ap[:, 7:10])
            nc.sync.dma_start(xt_sb[:, 10:13], xt_ap[:, 10:13])
            nc.gpsimd.dma_start(xt_sb[:, 13:16], xt_ap[:, 13:16])

            def mm(psum, pslot, ti, kc, w_sb):
                nc.tensor.matmul(
                    psum[:, pslot, :],
                    xt_sb[:, ti, kc : kc + kstep, :],
                    w_sb[:, kc : kc + kstep, :],
                    start=(kc == 0),
                    stop=(kc + kstep == KC),
                    perf_mode=pmode,
                )

            # pairs of tiles share a 2-bank PSUM tile and one evac op each;
            # head (pairs 0-1): XA phased by contraction pair so tile 0
            # gates on only the first half of a, XB deferred until bm lands
            head = {}
            for pi in range(2):
                z_new = zpair_pool.tile([P, 2, 2, C], mm_dt, tag="zpair")
                xa_new = xa_pool.tile([P, 2, FD], f32, tag="xa")
                xb_new = xb_pool.tile([P, 2, FD], f32, tag="xb")
                head[pi] = (z_new, xa_new, xb_new)
            for kc in range(0, KC, kstep):
                for tj in range(4):
                    mm(head[tj // 2][1], tj % 2, tj, kc, a_sb)
            for pi in range(2):
                z_pair, xa_ps, xb_ps = head[pi]
                nc.vector.tensor_copy(z_pair[:, :, 0, :], xa_ps[:])
            for pi in range(2):
                z_pair, xa_ps, xb_ps = head[pi]
                for j in range(2):
                    for kc in range(0, KC, kstep):
                        mm(xb_ps, j, 2 * pi + j, kc, bm_sb)
                nc.scalar.activation(
                    out=z_pair[:, :, 1, :],
                    in_=xb_ps[:],
                    func=mybir.ActivationFunctionType.Copy,
                )
                [nc.gpsimd, nc.scalar][pi % 2].dma_start(
                    z_ap[:, 2 * pi : 2 * pi + 2], z_pair[:]
                )

            for pi in range(2, NT // 2):
                t0 = 2 * pi
                z_pair = zpair_pool.tile([P, 2, 2, C], mm_dt, tag="zpair")
                xa_ps = xa_pool.tile([P, 2, FD], f32, tag="xa")
                xb_ps = xb_pool.tile([P, 2, FD], f32, tag="xb")
                for j in range(2):
                    for kc in range(0, KC, kstep):
                        mm(xa_ps, j, t0 + j, kc, a_sb)
                nc.vector.tensor_copy(z_pair[:, :, 0, :], xa_ps[:])
                if pi == NT // 2 - 1:
                    # tail: ship the XA halves as soon as the pair CAST is
                    # done, then per-tile XB evacs (ACT/DVE) with 64KB
                    # half-drains so the final transfer is minimal
                    nc.sync.dma_start(z_ap[:, t0, 0:1], z_pair[:, 0, 0:1])
                    nc.sync.dma_start(z_ap[:, t0 + 1, 0:1], z_pair[:, 1, 0:1])
                    for j in range(2):
                        for kc in range(0, KC, kstep):
                            mm(xb_ps, j, t0 + j, kc, bm_sb)
                    nc.scalar.activation(
                        out=z_pair[:, 0, 1, :],
                        in_=xb_ps[:, 0, :],
                        func=mybir.ActivationFunctionType.Copy,
                    )
                    nc.sync.dma_start(z_ap[:, t0, 1:2], z_pair[:, 0, 1:2])
                    nc.vector.tensor_copy(z_pair[:, 1, 1, :], xb_ps[:, 1, :])
                    nc.sync.dma_start(z_ap[:, t0 + 1, 1:2], z_pair[:, 1, 1:2])
                else:
                    for j in range(2):
                        for kc in range(0, KC, kstep):
                            mm(xb_ps, j, t0 + j, kc, bm_sb)
                    nc.scalar.activation(
                        out=z_pair[:, :, 1, :],
                        in_=xb_ps[:],
                        func=mybir.ActivationFunctionType.Copy,
                    )
                    [nc.gpsimd, nc.scalar][pi % 2].dma_start(
                        z_ap[:, t0 : t0 + 2], z_pair[:]
                    )

    nc.compile()
    return nc


def kernel(x, neighbors, Wq, bq, Wk, bk, Wv, bv, rel_bias, Wo, bo):
    from concourse.bass_utils import run_bass_kernel_spmd

    x = np.asarray(x, dtype=np.float32)
    Wq = np.asarray(Wq, dtype=np.float32)
    Wk = np.asarray(Wk, dtype=np.float32)
    Wv = np.asarray(Wv, dtype=np.float32)
    Wo = np.asarray(Wo, dtype=np.float32)
    bq = np.asarray(bq, dtype=np.float32)
    bk = np.asarray(bk, dtype=np.float32)
    bv = np.asarray(bv, dtype=np.float32)
    bo = np.asarray(bo, dtype=np.float32)
    rel_bias = np.asarray(rel_bias, dtype=np.float32)
    nbr = np.asarray(neighbors).reshape(N, -1)[:, 0].astype(np.int64)
    nbr2 = nbr[nbr]

    mm_np = _np_dt(MM_DT)

    # host-side weight folding (tiny)
    A = (Wq.T @ Wk).astype(np.float32)            # [C, C]
    Bm = (Wv.T @ Wo.T).astype(np.float32)         # [C, C]
    beta = (Wo @ bv + bo).astype(np.float32)      # [C]
    u = (Wq.T @ bk).astype(np.float32)
    v = (Wk.T @ bq).astype(np.float32)
    const = float(bq @ bk) + float(rel_bias[0, 0])

    key = MM_DT
    if key not in _CACHE:
        _CACHE[key] = _build_program(key)
    nc = _CACHE[key]

    def tile_T(t):  # [N, C] -> [P, NT, KC, P] (x^T pre-tiled per partition)
        return np.ascontiguousarray(
            t.reshape(NT, P, KC, P).transpose(3, 0, 2, 1)
        )

    A_t = np.ascontiguousarray(A.reshape(KC, P, C).transpose(1, 0, 2)).astype(mm_np)
    Bm_t = np.ascontiguousarray(Bm.reshape(KC, P, C).transpose(1, 0, 2)).astype(mm_np)

    in_maps = [
        {"xt": tile_T(x[b]).astype(mm_np), "a": A_t, "bm": Bm_t} for b in range(B)
    ]

    res = run_bass_kernel_spmd(
        nc,
        in_maps,
        core_ids=list(range(B)),
        trace=_TRACE["enabled"],
        trace_cores=_TRACE["trace_cores"],
    )
    _TRACE["last"] = res

    # host-side O(N*C) epilogue, exact in f32:
    # y[n] = w1[n]*(XB[nbr2[n]] + beta) + w0[n]*S'',  S'' = sxg@B + N*beta
    xg = x[:, nbr, :]                             # [B, N, C]
    sbias = x @ u + xg @ v + const                # [B, N]
    S2pp = xg.sum(axis=1) @ Bm + float(N) * beta  # [B, C] = S''

    y = np.empty((B, N, C), dtype=np.float32)
    for b in range(B):
        z = res.results[b]["z"].astype(np.float32)  # [P, NT, 2, C]
        z = z.transpose(1, 0, 2, 3)                 # [NT, P, 2, C]
        XA = np.ascontiguousarray(z[:, :, 0, :]).reshape(N, C)
        XB = np.ascontiguousarray(z[:, :, 1, :]).reshape(N, C)
        s = np.einsum("nc,nc->n", XA, xg[b]) + sbias[b]
        t = s / np.sqrt(C, dtype=np.float32)
        e = np.exp(t)
        w0 = 1.0 / (e + (N - 1))
        w1 = 1.0 - N * w0
        y[b] = w1[:, None] * (XB[nbr2] + beta[None, :]) + w0[:, None] * S2pp[b][None, :]
    return y
